# revision 1
# baseline (speedup 1.0000x reference)
"""MoD (mixture-of-depths) attention Bass kernel for Trainium2, 8 NeuronCores.

Problem: B=4, L=4096, D=1024, H=16, HD=64, K=1024 (top-25% tokens per row).
  scores = x @ w_router + b ; idx = top_k(scores, 1024) (desc order)
  xs = x[idx]; causal attention over score-ordered subsequence; out = x with
  selected rows replaced by attention output.

Sharding: core pair (2b, 2b+1) handles batch row b. Within a pair the 16
heads are split 8/8 (tensor parallel). Each core computes half the router
scores (AllGather within pair), full top-k redundantly, gathers xs via
indirect DMA, computes QKV for its 8 heads, causal attention, and a partial
out-projection over its 512 e-dims; a pair ReduceScatter(add) yields each
core's 512-token half of the final [1024, 1024] attention output.
Host reassembles: out[b] = x[b].copy(); out[b][idx] = concat(halves).
"""

import numpy as np

import concourse.bass as bass
import concourse.bacc as bacc
import concourse.mybir as mybir
import concourse.tile as tile
from concourse import library_config
from concourse.tile_rust import add_dep_helper
from concourse.bass import IndirectOffsetOnAxis
from concourse.bass_utils import run_bass_kernel_spmd
from concourse.library_overlay import lower_extended_insts

F32 = mybir.dt.float32
I32 = mybir.dt.int32
AF = mybir.ActivationFunctionType
OP = mybir.AluOpType

B, L, D = 4, 4096, 1024
H, HD = 16, 64
K = 1024
SCALE = 1.0 / 8.0
NEG = -1.0e30
EH = 512          # e-dims per core (8 heads)
NH_OWN = 8        # heads per core
N_TC = 8          # token chunks of 128 (K = 1024)
N_EBLK = 4        # e-blocks of 128 per core


def _consts():
    """Inline constant tensors (baked into the NEFF, DMA'd at load time)."""
    c = {}
    c["identity"] = np.eye(128, dtype=np.float32)
    # causal mask accumulators for S^T tiles [tk=128, tq=512]:
    # tile (m, n) crossing has delta = m*128 - n*512 in {0,128,256,384};
    # invalid (tk > tq) iff p + delta > f  -> add -1e30 there.
    u = np.zeros((4, 128, 512), dtype=np.float32)
    p = np.arange(128)[:, None]
    f = np.arange(512)[None, :]
    for di in range(4):
        u[di] = np.where(p + di * 128 > f, NEG, 0.0).astype(np.float32)
    c["umask"] = u
    # compaction index iota over [16, 256]: value = p*256 + f (fp32)
    c["iota16"] = (np.arange(16)[:, None] * 256 + np.arange(256)[None, :]).astype(
        np.float32
    )
    c["negones16"] = np.full((16, 256), -1.0, dtype=np.float32)
    # rank decomposition consts
    c["cbase"] = np.tile((np.arange(8) * 128).astype(np.float32), (128, 1))
    c["cbase_hi"] = c["cbase"] + 128.0
    c["cidx"] = np.tile(np.arange(8).astype(np.float32), (128, 1))
    c["pcol"] = np.tile(np.arange(128).astype(np.float32), (128, 1))
    return c


def _quantile_for(n_valid, k_adj):
    # kth_largest: k_adj = floor(omq * (n_valid - 1) / 2^32); out[1] = desc[k_adj + 1]
    return 1.0 - (k_adj + 0.5) / (n_valid - 1)


def build_program(n_cores=8, percore_shapes=False):
    """Builds the SPMD Bass program (same program on all cores; per-core
    behavior comes only from per-core input data). n_cores=1 builds the
    collective-free single-core variant (for simulation): full scores on the
    one core and no ReduceScatter (ypart is the output)."""
    spmd = n_cores > 1
    L_OWN = L // 2 if (spmd or percore_shapes) else L
    NSC = L_OWN // 128  # score tiles

    nc = bacc.Bacc("TRN2", num_devices=n_cores, debug=False)

    # ---- I/O ----
    x_row = nc.dram_tensor("x_row", [L, D], F32, kind="ExternalInput")
    x_sc = nc.dram_tensor("x_sc", [L_OWN, D], F32, kind="ExternalInput")
    w_rt = nc.dram_tensor("w_rt", [1, D], F32, kind="ExternalInput")
    b_rt = nc.dram_tensor("b_rt", [1, 1], F32, kind="ExternalInput")
    # wq/wk tiled host-side: [eblk, dblk, 128d, 128e]
    wq_t = nc.dram_tensor("wq_t", [N_EBLK, 8, 128, 128], F32, kind="ExternalInput")
    wk_t = nc.dram_tensor("wk_t", [N_EBLK, 8, 128, 128], F32, kind="ExternalInput")
    wv_o = nc.dram_tensor("wv_o", [D, EH], F32, kind="ExternalInput")
    wo_o = nc.dram_tensor("wo_o", [EH, D], F32, kind="ExternalInput")

    y_out_rows = K // 2 if (spmd or percore_shapes) else K
    y_out = nc.dram_tensor("y_out", [y_out_rows, D], F32, kind="ExternalOutput")
    idx_out = nc.dram_tensor("idx_out", [K], I32, kind="ExternalOutput")

    # ---- internal DRAM ----
    s_half_d = nc.dram_tensor("s_half_d", [L_OWN], F32, kind="Internal")
    if spmd or percore_shapes:
        s_full_d = nc.dram_tensor("s_full_d", [L], F32, kind="Internal")
        ypart_d = nc.dram_tensor("ypart_d", [K, D], F32, kind="Internal")
        y_red_d = nc.dram_tensor("y_red_d", [K // 2, D], F32, kind="Internal")
    else:
        s_full_d = s_half_d
    flat_v_d = nc.dram_tensor("flat_v_d", [1, K], F32, kind="Internal")
    rden_d = nc.dram_tensor("rden_d", [16, 512], F32, kind="Internal")

    consts = {k: nc.inline_tensor(v, name=f"c_{k}") for k, v in _consts().items()}

    PAIRS = [[2 * i, 2 * i + 1] for i in range(max(n_cores // 2, 1))]

    with tile.TileContext(nc) as tc:
        with (
            tc.tile_pool(name="const", bufs=1) as cpool,
            tc.tile_pool(name="ps", bufs=4, space="PSUM") as psp,
            tc.tile_pool(name="pst", bufs=2, space="PSUM") as pstp,
            tc.tile_pool(name="psb", bufs=2, space="PSUM") as psb,
        ):
            # ---------- constants to SBUF ----------
            ident = cpool.tile([128, 128], F32)
            nc.sync.dma_start(ident[:], consts["identity"][:])
            umask = cpool.tile([128, 4, 512], F32)
            for di in range(4):
                nc.sync.dma_start(umask[:, di, :], consts["umask"][di])
            cbase = cpool.tile([128, 8], F32)
            nc.sync.dma_start(cbase[:], consts["cbase"][:])
            cbase_hi = cpool.tile([128, 8], F32)
            nc.sync.dma_start(cbase_hi[:], consts["cbase_hi"][:])
            cidx = cpool.tile([128, 8], F32)
            nc.sync.dma_start(cidx[:], consts["cidx"][:])
            pcol = cpool.tile([128, 128], F32)
            nc.sync.dma_start(pcol[:], consts["pcol"][:])

            # phase-scoped activation tensors (manual release in sequence)
            actp1 = tc.alloc_tile_pool(name="actp1", bufs=1)
            xsT = actp1.tile([128, 8, K], F32, tag="xsT")
            i_sort_i = cpool.tile([128, 8], I32, tag="isrt")

            # ---------- phase A: scores, top-k, gather, transposes ----------
            with (
                tc.tile_pool(name="sa", bufs=1) as spool,
                tc.tile_pool(name="sca", bufs=2) as scpool,
            ):
                iota16 = spool.tile([16, 256], F32)
                nc.sync.dma_start(iota16[:], consts["iota16"][:])
                neg16 = spool.tile([16, 256], F32)
                nc.sync.dma_start(neg16[:], consts["negones16"][:])
                w_rep = spool.tile([128, D], F32)
                nc.sync.dma_start(w_rep[:], w_rt[:].to_broadcast((128, D)))
                b_bc = spool.tile([128, 1], F32)
                nc.sync.dma_start(b_bc[:], b_rt[:].to_broadcast((128, 1)))

                # scores (own half): tile j holds x rows {p*NSC + j} so the
                # score vector lands p-major => contiguous DRAM store.
                s_half = spool.tile([128, NSC], F32)
                x_sc_v = x_sc[:].rearrange("(p j) d -> j p d", j=NSC)
                for j in range(NSC):
                    xt = scpool.tile([128, D], F32, tag="xsc")
                    nc.sync.dma_start(xt[:], x_sc_v[j])
                    prod = scpool.tile([128, D], F32, tag="prod")
                    nc.vector.tensor_tensor(
                        out=prod[:], in0=xt[:], in1=w_rep[:], op=OP.mult
                    )
                    acc_scr = scpool.tile([128, D], F32, tag="accscr")
                    nc.scalar.activation(
                        acc_scr[:], prod[:], AF.Copy,
                        accum_out=s_half[:, j : j + 1],
                    )
                nc.vector.tensor_scalar(
                    s_half[:], s_half[:], b_bc[:], None, op0=OP.add
                )
                nc.sync.dma_start(
                    s_half_d[:].rearrange("(p j) -> p j", j=NSC), s_half[:]
                )

                # all-gather scores within pair
                if percore_shapes:
                    nc.sync.dma_start(s_full_d[0:L_OWN], s_half_d[:])
                    nc.sync.dma_start(s_full_d[L_OWN:L], s_half_d[:])
                if spmd:
                    nc.gpsimd.collective_compute(
                        "AllGather",
                        OP.bypass,
                        replica_groups=PAIRS,
                        ins=[s_half_d[:]],
                        outs=[s_full_d[:]],
                    )

                s_sb = spool.tile([128, 32], F32)
                nc.sync.dma_start(
                    s_sb[:], s_full_d[:].rearrange("(p f) -> p f", f=32)
                )
                s16 = spool.tile([16, 256], F32)
                nc.sync.dma_start(
                    s16[:], s_full_d[:].rearrange("(p f) -> p f", f=256)
                )

                # exact threshold T (1024th largest) via 3 masked rounds
                s_work = spool.tile([128, 32], F32)
                nc.vector.tensor_copy(s_work[:], s_sb[:])
                negtile = spool.tile([128, 32], F32)
                nc.vector.memset(negtile[:], NEG)
                kth = spool.tile([1, 2], F32)
                t_bc = spool.tile([128, 1], F32)
                rounds = [(4096, 508), (3586, 508), (3076, 2)]
                for r, (n_valid, k_adj) in enumerate(rounds):
                    nc.gpsimd.kth_largest(
                        kth[:], s_work[:], 32, 510,
                        quantile=_quantile_for(n_valid, k_adj),
                    )
                    nc.gpsimd.partition_broadcast(t_bc[:], kth[0:1, 1:2])
                    if r < len(rounds) - 1:
                        ge = spool.tile([128, 32], mybir.dt.uint8, tag="gemask")
                        nc.vector.tensor_scalar(
                            ge[:], s_work[:], t_bc[:], None, op0=OP.is_ge
                        )
                        nc.vector.copy_predicated(s_work[:], ge[:], negtile[:])

                # compact selected values & original indices
                shifted = spool.tile([16, 256], F32)
                nc.vector.tensor_scalar(
                    shifted[:], s16[:], t_bc[0:16, :], None, op0=OP.subtract
                )
                mask16 = spool.tile([16, 256], mybir.dt.uint8)
                nc.vector.tensor_scalar(
                    mask16[:], shifted[:], 0.0, None, op0=OP.is_ge
                )
                idx16 = spool.tile([16, 256], F32)
                nc.vector.tensor_copy(idx16[:], neg16[:])
                nc.vector.copy_predicated(idx16[:], mask16[:], iota16[:])

                nf = spool.tile([1, 1], mybir.dt.uint32)
                v_comp = spool.tile([16, 64], F32)
                nc.gpsimd.sparse_gather(v_comp[:], shifted[:], num_found=nf[:])
                nf2 = spool.tile([1, 1], mybir.dt.uint32)
                i_comp = spool.tile([16, 64], F32)
                nc.gpsimd.sparse_gather(i_comp[:], idx16[:], num_found=nf2[:])

                # reshape compacted streams: [16, 64] -> [64, 16] -> [8, 128]
                v64_ps = psb.tile([64, 16], F32, tag="tsm")
                nc.tensor.transpose(v64_ps[:], v_comp[:], ident[0:16, 0:16])
                v64 = spool.tile([64, 16], F32)
                nc.vector.tensor_copy(v64[:], v64_ps[:])
                i64_ps = psb.tile([64, 16], F32, tag="tsm")
                nc.tensor.transpose(i64_ps[:], i_comp[:], ident[0:16, 0:16])
                i64 = spool.tile([64, 16], F32)
                nc.vector.tensor_copy(i64[:], i64_ps[:])

                s8v = spool.tile([8, 128], F32)
                nc.sync.dma_start(s8v[:], v64[:])
                s8i = spool.tile([8, 128], F32)
                nc.sync.dma_start(s8i[:], i64[:])

                # flat [1, 1024] via DRAM, then broadcast to [128, 1024]
                nc.sync.dma_start(flat_v_d[:], s8v[:])
                rep = spool.tile([128, K], F32)
                nc.sync.dma_start(rep[:], flat_v_d[:].to_broadcast((128, K)))

                # per-chunk scalars [128, 8]
                vch_ps = psb.tile([128, 8], F32, tag="tsm")
                nc.tensor.transpose(vch_ps[:], s8v[:], ident[0:8, 0:8])
                v_ch = spool.tile([128, 8], F32)
                nc.vector.tensor_copy(v_ch[:], vch_ps[:])
                ich_ps = psb.tile([128, 8], F32, tag="tsm")
                nc.tensor.transpose(ich_ps[:], s8i[:], ident[0:8, 0:8])
                i_ch = spool.tile([128, 8], F32)
                nc.vector.tensor_copy(i_ch[:], ich_ps[:])

                # ranks among selected
                ranks = spool.tile([128, 8], F32)
                for c in range(8):
                    rankscr = scpool.tile([128, K], F32, tag="rankscr")
                    nc.vector.tensor_tensor(
                        out=rankscr[:], in0=rep[:],
                        in1=v_ch[:, c : c + 1].to_broadcast((128, K)),
                        op=OP.is_gt,
                    )
                    rankscr2 = scpool.tile([128, K], F32, tag="rankscr2")
                    nc.scalar.activation(
                        rankscr2[:], rankscr[:], AF.Copy,
                        accum_out=ranks[:, c : c + 1],
                    )

                # permutation: i_sorted[p, c] = original idx with rank c*128+p
                isort_ps = psb.tile([128, 8], F32, tag="tsm")
                for c in range(8):
                    rank_c = ranks[:, c : c + 1]
                    ge_lo = spool.tile([128, 8], F32, tag="rge")
                    nc.vector.tensor_tensor(
                        out=ge_lo[:], in0=rank_c.to_broadcast((128, 8)),
                        in1=cbase[:], op=OP.is_ge,
                    )
                    lt_hi = spool.tile([128, 8], F32, tag="rlt")
                    nc.vector.tensor_tensor(
                        out=lt_hi[:], in0=rank_c.to_broadcast((128, 8)),
                        in1=cbase_hi[:], op=OP.is_lt,
                    )
                    r_ci = spool.tile([128, 8], F32, tag="rci")
                    nc.vector.tensor_tensor(
                        out=r_ci[:], in0=ge_lo[:], in1=lt_hi[:], op=OP.mult
                    )
                    cdiv = spool.tile([128, 1], F32, tag="cdiv")
                    tmp8 = spool.tile([128, 8], F32, tag="tmp8")
                    nc.vector.tensor_tensor(
                        out=tmp8[:], in0=r_ci[:], in1=cidx[:], op=OP.mult
                    )
                    nc.vector.reduce_sum(
                        cdiv[:], tmp8[:], axis=mybir.AxisListType.X
                    )
                    rmod = spool.tile([128, 1], F32, tag="rmod")
                    nc.vector.scalar_tensor_tensor(
                        out=rmod[:], in0=cdiv[:], scalar=-128.0, in1=rank_c,
                        op0=OP.mult, op1=OP.add,
                    )
                    m_oh = scpool.tile([128, 128], F32, tag="moh")
                    nc.vector.tensor_tensor(
                        out=m_oh[:], in0=rmod[:].to_broadcast((128, 128)),
                        in1=pcol[:], op=OP.is_equal,
                    )
                    m_ci = scpool.tile([128, 128], F32, tag="mci")
                    nc.vector.tensor_scalar(
                        m_ci[:], m_oh[:], i_ch[:, c : c + 1], None, op0=OP.mult
                    )
                    nc.tensor.matmul(
                        isort_ps[:], m_ci[:], r_ci[:],
                        start=(c == 0), stop=(c == 7),
                    )

                nc.vector.tensor_copy(i_sort_i[:], isort_ps[:])
                i_sort_f = spool.tile([128, 8], F32)
                nc.vector.tensor_copy(i_sort_f[:], isort_ps[:])

                # idx_out [1024] in token order (t = c*128 + p)
                it_ps = psb.tile([8, 128], F32, tag="tsm")
                nc.tensor.transpose(it_ps[:], i_sort_f[:], ident[:])
                it_sb = spool.tile([8, 128], I32)
                nc.vector.tensor_copy(it_sb[:], it_ps[:])
                nc.sync.dma_start(
                    idx_out[:].rearrange("(c p) -> c p", p=128), it_sb[:]
                )

                # gather xs [128, 8, 1024] (t = c*128 + p), then transpose
                xs = spool.tile([128, N_TC, D], F32, tag="xs")
                for c in range(N_TC):
                    nc.gpsimd.indirect_dma_start(
                        out=xs[:, c, :],
                        out_offset=None,
                        in_=x_row[:],
                        in_offset=IndirectOffsetOnAxis(
                            ap=i_sort_i[:, c : c + 1], axis=0
                        ),
                    )
                for dblk in range(8):
                    for c in range(N_TC):
                        tp = pstp.tile([128, 128], F32, tag="ps128")
                        nc.tensor.transpose(
                            tp[:], xs[:, c, dblk * 128 : (dblk + 1) * 128],
                            ident[:],
                        )
                        nc.any.tensor_copy(
                            xsT[:, dblk, c * 128 : (c + 1) * 128], tp[:]
                        )

            # ---------- Q^T, K^T [eblk][128e, 1024t]; V [tc][128t, 8h, 65] ----------
            actp2 = tc.alloc_tile_pool(name="actp2", bufs=1)
            qT = actp2.tile([128, N_EBLK, K], F32, tag="qT")
            kT = actp2.tile([128, N_EBLK, K], F32, tag="kT")
            v_sb = actp2.tile([128, N_TC, NH_OWN, 65], F32, tag="v")
            wpool = tc.alloc_tile_pool(name="wpool", bufs=2)
            for eblk in range(N_EBLK):
                wq_sb = wpool.tile([128, 8, 128], F32, tag="wq")
                nc.sync.dma_start(wq_sb[:], wq_t[eblk].rearrange("k p e -> p k e"))
                wk_sb = wpool.tile([128, 8, 128], F32, tag="wk")
                nc.sync.dma_start(wk_sb[:], wk_t[eblk].rearrange("k p e -> p k e"))
                for tch in range(2):
                    tsl = bass.ts(tch, 512)
                    pq = psp.tile([128, 512], F32, tag="ps512")
                    pk = psp.tile([128, 512], F32, tag="ps512")
                    for dblk in range(8):
                        nc.tensor.matmul(
                            pq[:], wq_sb[:, dblk, :], xsT[:, dblk, tsl],
                            start=(dblk == 0), stop=(dblk == 7),
                        )
                    for dblk in range(8):
                        nc.tensor.matmul(
                            pk[:], wk_sb[:, dblk, :], xsT[:, dblk, tsl],
                            start=(dblk == 0), stop=(dblk == 7),
                        )
                    nc.any.tensor_copy(qT[:, eblk, tsl], pq[:])
                    nc.any.tensor_copy(kT[:, eblk, tsl], pk[:])

            wpool.release()
            wvp = tc.alloc_tile_pool(name="wvp", bufs=1)
            wv_all = wvp.tile([128, 8, 512], F32, tag="wv")
            nc.sync.dma_start(
                wv_all[:], wv_o[:].rearrange("(k p) e -> p k e", p=128)
            )
            v_one = wvp.tile([128, N_TC * NH_OWN], F32, tag="vone")
            nc.vector.memset(v_one[:], 1.0)
            nc.vector.tensor_copy(
                v_sb[:, :, :, 64],
                v_one[:].rearrange("p (t h) -> p t h", t=N_TC),
            )
            for tc_i in range(N_TC):
                pv = psp.tile([128, 512], F32, tag="ps512")
                for dblk in range(8):
                    nc.tensor.matmul(
                        pv[:],
                        xsT[:, dblk, tc_i * 128 : (tc_i + 1) * 128],
                        wv_all[:, dblk, :],
                        start=(dblk == 0), stop=(dblk == 7),
                    )
                nc.any.tensor_copy(
                    v_sb[:, tc_i, :, 0:64],
                    pv[:].rearrange("p (h e) -> p h e", h=8),
                )

            # ---------- attention per head; O^T rows hh*64..hh*64+63 ----------
            wvp.release()
            actp3 = tc.alloc_tile_pool(name="actp3", bufs=1)
            oT = actp3.tile([128, N_EBLK, K], F32, tag="oT")
            expp = tc.alloc_tile_pool(name="expp", bufs=3)
            for eblk in range(N_EBLK):
                for sub in range(2):
                    hh = eblk * 2 + sub
                    esl = slice(sub * 64, sub * 64 + 64)
                    for n in range(2):
                        tql = bass.ts(n, 512)
                        po = psp.tile([65, 512], F32, tag="ps512")
                        n_m = 4 * n + 4
                        for m in range(n_m):
                            ps_s = psp.tile([128, 512], F32, tag="ps512")
                            crossing = m * 128 + 127 > n * 512
                            if crossing:
                                di = m - 4 * n
                                nc.tensor.matmul(
                                    ps_s[:], ident[:], umask[:, di, :],
                                    start=True, stop=False,
                                )
                            nc.tensor.matmul(
                                ps_s[:],
                                kT[esl, eblk, m * 128 : (m + 1) * 128],
                                qT[esl, eblk, tql],
                                start=not crossing, stop=True,
                                tile_position=(sub * 64, 0),
                            )
                            es = expp.tile([128, 512], F32, tag="es")
                            nc.scalar.activation(
                                es[:], ps_s[:], AF.Exp, scale=SCALE
                            )
                            nc.tensor.matmul(
                                po[:], v_sb[:, m, hh, :], es[:],
                                start=(m == 0), stop=(m == n_m - 1),
                            )
                        # normalize rows 0..63 by row 64
                        r_row = expp.tile([1, 512], F32, tag="rrow")
                        nc.vector.reciprocal(r_row[:], po[64:65, :])
                        slot = hh * 2 + n
                        nc.sync.dma_start(rden_d[slot : slot + 1, :], r_row[:])
                        r_bc = expp.tile([64, 512], F32, tag="rbc")
                        nc.sync.dma_start(
                            r_bc[:],
                            rden_d[slot : slot + 1, :].to_broadcast((64, 512)),
                        )
                        nc.vector.tensor_tensor(
                            out=oT[sub * 64 : sub * 64 + 64, eblk, tql],
                            in0=po[0:64, :], in1=r_bc[:], op=OP.mult,
                        )

            # ---------- out-projection partial: ypart[t, :] ----------
            wop = tc.alloc_tile_pool(name="wop", bufs=1)
            wo_all = wop.tile([128, N_EBLK, D], F32, tag="wo")
            nc.sync.dma_start(
                wo_all[:], wo_o[:].rearrange("(k p) d -> p k d", p=128)
            )
            ydst = ypart_d if (spmd or percore_shapes) else y_out
            for tc_i in range(N_TC):
                for dc in range(2):
                    py = psp.tile([128, 512], F32, tag="ps512")
                    for eblk in range(N_EBLK):
                        nc.tensor.matmul(
                            py[:],
                            oT[:, eblk, tc_i * 128 : (tc_i + 1) * 128],
                            wo_all[:, eblk, dc * 512 : (dc + 1) * 512],
                            start=(eblk == 0), stop=(eblk == N_EBLK - 1),
                        )
                    y_sb = expp.tile([128, 512], F32, tag="ysb")
                    nc.any.tensor_copy(y_sb[:], py[:])
                    nc.sync.dma_start(
                        ydst[tc_i * 128 : (tc_i + 1) * 128,
                             dc * 512 : (dc + 1) * 512],
                        y_sb[:],
                    )

            wop.release()
            expp.release()
            actp3.release()
            actp2.release()
            actp1.release()

            if percore_shapes:
                nc.sync.dma_start(y_out[:], ypart_d[0 : K // 2, :])
            if spmd:
                nc.gpsimd.collective_compute(
                    "ReduceScatter",
                    OP.add,
                    replica_groups=PAIRS,
                    ins=[ypart_d[:]],
                    outs=[y_red_d[:]],
                )
                nc.sync.dma_start(y_out[:], y_red_d[:])

    nc.compile()
    return nc


_NC_CACHE = {}


def _get_nc(n_cores=8):
    if n_cores not in _NC_CACHE:
        _NC_CACHE[n_cores] = build_program(n_cores)
    return _NC_CACHE[n_cores]


def _weight_tiles(w_half):
    # [1024, 512] -> [eblk, dblk, 128d, 128e]
    return np.ascontiguousarray(
        w_half.reshape(8, 128, 4, 128).transpose(2, 0, 1, 3)
    )


def _build_in_maps(inputs):
    x = np.ascontiguousarray(np.asarray(inputs["x"], np.float32))
    w_router = np.asarray(inputs["w_router"], np.float32)
    b_router = np.asarray(inputs["b_router"], np.float32)
    wq = np.asarray(inputs["wq"], np.float32)
    wk = np.asarray(inputs["wk"], np.float32)
    wv = np.asarray(inputs["wv"], np.float32)
    wo = np.asarray(inputs["wo"], np.float32)

    in_maps = []
    for core in range(8):
        b = core // 2
        half = core % 2
        esl = slice(half * EH, (half + 1) * EH)
        in_maps.append(
            {
                "x_row": x[b],
                "x_sc": np.ascontiguousarray(x[b, half * 2048 : (half + 1) * 2048]),
                "w_rt": w_router.reshape(1, D),
                "b_rt": b_router.reshape(1, 1),
                "wq_t": _weight_tiles(wq[:, esl]),
                "wk_t": _weight_tiles(wk[:, esl]),
                "wv_o": np.ascontiguousarray(wv[:, esl]),
                "wo_o": np.ascontiguousarray(wo[esl, :]),
            }
        )
    return in_maps


def kernel(x, w_router, b_router, wq, wk, wv, wo):
    x = np.asarray(x, np.float32)
    nc = _get_nc(8)
    in_maps = _build_in_maps(
        dict(x=x, w_router=w_router, b_router=b_router, wq=wq, wk=wk, wv=wv, wo=wo)
    )
    res = run_bass_kernel_spmd(nc, in_maps, core_ids=list(range(8)))
    out = x.copy()
    for b in range(B):
        idx = res.results[2 * b]["idx_out"].astype(np.int64)
        y = np.concatenate(
            [res.results[2 * b]["y_out"], res.results[2 * b + 1]["y_out"]], axis=0
        )
        out[b][idx] = y
    return out

